# revision 1
# baseline (speedup 1.0000x reference)
"""Trainium2 Bass kernel for nn_ConditionalDisCoLoss.

loss = BCEWithLogits(inputs, targets)
     + dCor_masked(sigmoid(inputs), spectators, mask=spectators>=0.5)

Reformulation (no centered n x n matrices):
  p = sigmoid(x), m = (s >= 0.5), c = max(sum m, 1)
  A_i = sum_j m_i m_j |p_i - p_j|,  B_i likewise for s
  Sxy = sum_ij m_i m_j |p_i-p_j||s_i-s_j|
  Sxx = 2c*sum(m p^2) - 2(sum m p)^2   (closed form), Syy likewise
  Vxy = Sxy - (2/c) sum A_i B_i + (sum A)(sum B)/c^2  (and Vxx, Vyy)
  dcor = sqrt(max(Vxy,eps'))/sqrt(...)   with the reference's eps placement

Distribution + symmetry: the pair matrix is symmetric, so only j-bands
at or above each row's band are computed.  Global i-tiles (128 rows) are
dealt round-robin: core k owns i-tiles {8*it + k}, whose band is exactly
`it`, so every core runs the SAME program (jt in [it, 8)) on different
gathered row data - 36 of 64 tiles each.

Per tile [128 x 1024]:
 - PE: masked pairwise diffs D1 = m_i m_j (p_i - p_j) via K=4 bf16
   matmuls (bf16 hi+lo split of p keeps ~1e-7 element accuracy)
 - ACT: U = |D1| (bf16) + fused row-sum accum (A row-part); part of |D2|
 - DVE: rest of |D2| via abs_max + fused accum; product U*V with fused
   row-sum accum (Sxy partials)
 - PE: column sums of U,V for strictly-upper tiles (the transposed
   pairs' row sums) via [128,128]^T @ ones accumulated in one PSUM bank
Host combines per-core partial A/B vectors and scalars in float64.
"""

import numpy as np
from contextlib import ExitStack

import concourse.bass as bass
import concourse.bacc as bacc
import concourse.tile as tile
from concourse import mybir
from concourse.bass_utils import run_bass_kernel_spmd

N = 8192
NCORES = 8
STRIP = N // NCORES      # 1024 rows per core (gathered, not contiguous)
P = 128
JT = 1024                # j-tile width (one band = one j-tile)
NB = N // JT             # 8 bands
NIT = STRIP // P         # 8 i-tiles per core; i-tile it sits in band it
F_FULL = N // P          # 64
F_STRIP = STRIP // P     # 8
SPLIT_ACT = 704          # columns of |D2| done on ACT; rest on DVE

F32 = mybir.dt.float32
BF16 = mybir.dt.bfloat16
F32R = mybir.dt.float32r
ALU = mybir.AluOpType
ACTF = mybir.ActivationFunctionType
AX = mybir.AxisListType

NOUT = 16
# partials slots: 5 sum(R_diag), 6 sum(m), 7 sum(m*p), 8 sum(m*p^2),
#                 9 sum(m*s), 10 sum(m*s^2), 11 sum(bce), 12 sum(R_upper)
NCOLP = 112  # colparts: 7 bands x (8 quarters A | 8 quarters B)


def _build():
    nc = bacc.Bacc("TRN2", target_bir_lowering=False, debug=False,
                   num_devices=NCORES, enable_asserts=False)

    x_full = nc.dram_tensor("x_full", [N, 1], F32, kind="ExternalInput")
    s_full = nc.dram_tensor("s_full", [N], F32, kind="ExternalInput")
    x_strip = nc.dram_tensor("x_strip", [STRIP, 1], F32, kind="ExternalInput")
    t_strip = nc.dram_tensor("t_strip", [STRIP, 1], F32, kind="ExternalInput")
    s_strip = nc.dram_tensor("s_strip", [STRIP], F32, kind="ExternalInput")
    out = nc.dram_tensor("partials", [NOUT], F32, kind="ExternalOutput")
    rowp = nc.dram_tensor("rowparts", [P, 16], F32, kind="ExternalOutput")
    colp = nc.dram_tensor("colparts", [P, NCOLP], F32, kind="ExternalOutput")

    with tile.TileContext(nc) as tc, ExitStack() as ctx:
        pre = ctx.enter_context(tc.tile_pool(name="pre", bufs=1))
        uvp = ctx.enter_context(tc.tile_pool(name="uv", bufs=3))
        accp = ctx.enter_context(tc.tile_pool(name="acc", bufs=2))
        psp = ctx.enter_context(tc.tile_pool(name="psp", bufs=3, space="PSUM"))
        psc = ctx.enter_context(tc.tile_pool(name="psc", bufs=1, space="PSUM"))

        # ---------- preprocessing: full vectors -> moving operands ----------
        xf = pre.tile([P, F_FULL], F32)
        sf = pre.tile([P, F_FULL], F32)
        nc.sync.dma_start(out=xf, in_=x_full.ap().rearrange("(p f) one -> p (f one)", p=P))
        nc.scalar.dma_start(out=sf, in_=s_full.ap().rearrange("(p f) -> p f", p=P))

        pf = pre.tile([P, F_FULL], F32)
        nc.scalar.activation(pf, xf, ACTF.Sigmoid)
        mf = pre.tile([P, F_FULL], F32)
        nc.vector.tensor_scalar(mf, sf, 0.5, None, ALU.is_ge)
        af = pre.tile([P, F_FULL], F32)
        nc.vector.tensor_tensor(out=af, in0=mf, in1=pf, op=ALU.mult)
        cf = pre.tile([P, F_FULL], F32)
        nc.vector.tensor_tensor(out=cf, in0=mf, in1=sf, op=ALU.mult)

        # moving operands (f32, fed to the PE as float32r via bitcast):
        # RA rows: m, a=m*p   RB rows: m, c=m*s
        RA = pre.tile([2, N], F32)
        RB = pre.tile([2, N], F32)
        for eng, dst, row, src in ((nc.sync, RA, 0, mf), (nc.scalar, RA, 1, af),
                                   (nc.sync, RB, 0, mf), (nc.scalar, RB, 1, cf)):
            eng.dma_start(out=dst[row:row + 1, :], in_=src)

        # ---------- preprocessing: gathered strip -> stationary operands ----------
        # [16, 64] layout: strip position s = p*64 + f (DMA-friendly 256B rows)
        PS, FS = 16, 64
        xs = pre.tile([PS, FS], F32)
        ts = pre.tile([PS, FS], F32)
        ss = pre.tile([PS, FS], F32)
        nc.sync.dma_start(out=xs, in_=x_strip.ap().rearrange("(p f) one -> p (f one)", p=PS))
        nc.scalar.dma_start(out=ts, in_=t_strip.ap().rearrange("(p f) one -> p (f one)", p=PS))
        nc.sync.dma_start(out=ss, in_=s_strip.ap().rearrange("(p f) -> p f", p=PS))

        ps_ = pre.tile([PS, FS], F32)
        nc.scalar.activation(ps_, xs, ACTF.Sigmoid)
        ms = pre.tile([PS, FS], F32)
        nc.vector.tensor_scalar(ms, ss, 0.5, None, ALU.is_ge)
        negm = pre.tile([PS, FS], F32)
        nc.vector.tensor_scalar(negm, ms, -1.0, None, ALU.mult)

        bs = pre.tile([PS, FS], F32)
        nc.vector.tensor_tensor(out=bs, in0=ms, in1=ps_, op=ALU.mult)
        ds = pre.tile([PS, FS], F32)
        nc.vector.tensor_tensor(out=ds, in0=ms, in1=ss, op=ALU.mult)

        # stationary operands: LA rows (b, -m), LB rows (d, -m)
        LA = pre.tile([2, STRIP], F32)
        LB = pre.tile([2, STRIP], F32)
        for eng, dst, row, src in ((nc.sync, LA, 0, bs), (nc.scalar, LA, 1, negm),
                                   (nc.sync, LB, 0, ds), (nc.scalar, LB, 1, negm)):
            eng.dma_start(out=dst[row:row + 1, :], in_=src)

        # ---------- O(n) scalar columns (strip tiles live on partitions 0:16,
        # rest of cat stays zero and drops out of the final ones-matmul) ----------
        cat = pre.tile([P, NOUT], F32)
        nc.vector.memset(cat, 0.0)
        junk_s = pre.tile([PS, FS], F32)

        nc.vector.tensor_reduce(cat[0:PS, 6:7], ms, AX.X, ALU.add)
        nc.vector.tensor_reduce(cat[0:PS, 7:8], bs, AX.X, ALU.add)
        nc.vector.scalar_tensor_tensor(out=junk_s, in0=bs, scalar=0.0,
                                       in1=ps_, op0=ALU.bypass, op1=ALU.mult,
                                       accum_out=cat[0:PS, 8:9])
        nc.vector.tensor_reduce(cat[0:PS, 9:10], ds, AX.X, ALU.add)
        junk_s2 = pre.tile([PS, FS], F32)
        nc.vector.scalar_tensor_tensor(out=junk_s2, in0=ds, scalar=0.0,
                                       in1=ss, op0=ALU.bypass, op1=ALU.mult,
                                       accum_out=cat[0:PS, 10:11])

        # ---------- main pass: tiles (it, jt) with jt >= it ----------
        ones = pre.tile([P, 1], BF16)
        nc.vector.memset(ones, 1.0)
        onesf = pre.tile([P, 1], F32)
        nc.vector.memset(onesf, 1.0)

        # per-tile column sums, rectangular [it][jt][16] layout (no PSUM
        # accumulation -- scheduler may reorder same-engine matmuls, so
        # every tile writes its own fresh column; reduced over it at the end)
        colacc = psc.tile([P, NIT, NB, 16], F32)

        AA = pre.tile([P, NIT], F32)
        BB = pre.tile([P, NIT], F32)
        RRd = pre.tile([P, NIT], F32)
        RRu = pre.tile([P, NIT], F32)
        nc.vector.memset(RRu, 0.0)

        for it in range(NIT):
            njt = NB - it
            Ap = accp.tile([P, NB], F32, tag="Ap")
            Bp = accp.tile([P, 2 * NB], F32, tag="Bp")
            Rp = accp.tile([P, NB], F32, tag="Rp")
            lA = LA[:, it * P:(it + 1) * P]
            lB = LB[:, it * P:(it + 1) * P]
            for jj in range(njt):
                jt = it + jj
                psA = psp.tile([P, JT], F32, tag="ps")
                psB = psp.tile([P, JT], F32, tag="ps")
                for h in range(JT // 512):
                    j0 = jt * JT + h * 512
                    nc.tensor.matmul(psA[:, h * 512:(h + 1) * 512],
                                     lhsT=lA.bitcast(F32R),
                                     rhs=RA[:, j0:j0 + 512].bitcast(F32R),
                                     start=True, stop=True)
                    nc.tensor.matmul(psB[:, h * 512:(h + 1) * 512],
                                     lhsT=lB.bitcast(F32R),
                                     rhs=RB[:, j0:j0 + 512].bitcast(F32R),
                                     start=True, stop=True)
                U = uvp.tile([P, JT], BF16, tag="U")
                V = uvp.tile([P, JT], F32, tag="V")
                nc.scalar.activation(U, psA, ACTF.Abs, accum_out=Ap[:, jj:jj + 1])
                nc.scalar.activation(V[:, 0:SPLIT_ACT], psB[:, 0:SPLIT_ACT], ACTF.Abs,
                                     accum_out=Bp[:, 2 * jj:2 * jj + 1])
                # |x| on DVE in 2 ops (only one PSUM operand allowed per op):
                # Vn = -psB_slice (PSUM->SBUF), then V2 = max(Vn, psB_slice)
                Vn = uvp.tile([P, JT - SPLIT_ACT], F32, tag="Vn")
                nc.vector.tensor_scalar(Vn, psB[:, SPLIT_ACT:JT], -1.0, None, ALU.mult)
                nc.vector.scalar_tensor_tensor(out=V[:, SPLIT_ACT:JT],
                                               in0=Vn, scalar=0.0,
                                               in1=psB[:, SPLIT_ACT:JT],
                                               op0=ALU.bypass, op1=ALU.max,
                                               accum_out=Bp[:, 2 * jj + 1:2 * jj + 2])
                W = uvp.tile([P, JT], F32, tag="W")
                nc.vector.scalar_tensor_tensor(out=W, in0=U, scalar=0.0,
                                               in1=V, op0=ALU.bypass, op1=ALU.mult,
                                               accum_out=Rp[:, jj:jj + 1])
                if jt > it:
                    # transposed pairs' row sums = column sums, via PE
                    for q in range(8):
                        nc.tensor.matmul(colacc[:, it, jt, q:q + 1],
                                         lhsT=U[:, q * P:(q + 1) * P], rhs=ones,
                                         start=True, stop=True)
                        nc.tensor.matmul(colacc[:, it, jt, q + 8:q + 9],
                                         lhsT=V[:, q * P:(q + 1) * P], rhs=onesf,
                                         start=True, stop=True)
            nc.vector.tensor_reduce(AA[:, it:it + 1], Ap[:, 0:njt], AX.X, ALU.add)
            nc.vector.tensor_reduce(BB[:, it:it + 1], Bp[:, 0:2 * njt], AX.X, ALU.add)
            nc.vector.tensor_copy(RRd[:, it:it + 1], Rp[:, 0:1])
            if njt > 1:
                nc.vector.tensor_reduce(RRu[:, it:it + 1], Rp[:, 1:njt], AX.X, ALU.add)

        # ---------- outputs ----------
        # BCE partial: relu(x) - x*t + softplus(-|x|) = relu - xt + ln(1+exp(-|x|))
        rx = pre.tile([PS, FS], F32)
        nc.vector.tensor_scalar(rx, xs, 0.0, None, ALU.max)
        xt = pre.tile([PS, FS], F32)
        nc.vector.tensor_tensor(out=xt, in0=xs, in1=ts, op=ALU.mult)
        axx = pre.tile([PS, FS], F32)
        nc.scalar.activation(axx, xs, ACTF.Abs)
        enx = pre.tile([PS, FS], F32)
        nc.scalar.activation(enx, axx, ACTF.Exp, scale=-1.0)
        sp = pre.tile([PS, FS], F32)
        nc.scalar.activation(sp, enx, ACTF.Ln, bias=1.0)
        t1 = pre.tile([PS, FS], F32)
        nc.vector.tensor_tensor(out=t1, in0=rx, in1=xt, op=ALU.subtract)
        t2 = pre.tile([PS, FS], F32)
        nc.vector.scalar_tensor_tensor(out=t2, in0=t1, scalar=0.0, in1=sp,
                                       op0=ALU.add, op1=ALU.add,
                                       accum_out=cat[0:PS, 11:12])

        nc.vector.tensor_reduce(cat[:, 5:6], RRd, AX.X, ALU.add)
        nc.vector.tensor_reduce(cat[:, 12:13], RRu, AX.X, ALU.add)

        pcat = psp.tile([NOUT, 1], F32, tag="ps")
        nc.tensor.matmul(pcat, lhsT=cat, rhs=onesf, start=True, stop=True)
        outt = pre.tile([NOUT, 1], F32)
        nc.scalar.copy(outt, pcat)
        nc.sync.dma_start(out=out.ap().rearrange("(a b) -> a b", b=1), in_=outt)

        rowt = pre.tile([P, 16], F32)
        nc.vector.tensor_copy(rowt[:, 0:8], AA)
        nc.vector.tensor_copy(rowt[:, 8:16], BB)
        nc.sync.dma_start(out=rowp.ap(), in_=rowt)

        # reduce per-tile column sums over it (strided AP: last dim = it)
        colt = pre.tile([P, NCOLP], F32)
        for jt in range(1, NB):
            for half in range(2):  # 0: A quarters, 1: B quarters
                src = colacc[:, 0:jt, jt, half * 8:(half + 1) * 8]
                src = src.rearrange("p i q -> p q i")
                nc.vector.tensor_reduce(
                    colt[:, (jt - 1) * 16 + half * 8:(jt - 1) * 16 + (half + 1) * 8],
                    src, AX.X, ALU.add)
        nc.scalar.dma_start(out=colp.ap(), in_=colt)

    nc.compile()
    return nc


_NC_CACHE = None


def _get_nc():
    global _NC_CACHE
    if _NC_CACHE is None:
        _NC_CACHE = _build()
    return _NC_CACHE


def _row_index(k):
    """Global row indices owned by core k (i-tiles 8*it + k)."""
    idx = []
    for it_ in range(NIT):
        t = 8 * it_ + k
        idx.append(np.arange(t * P, (t + 1) * P))
    return np.concatenate(idx)


def _make_in_maps(inputs, targets, spectators):
    x = np.ascontiguousarray(np.asarray(inputs, dtype=np.float32)).reshape(N, 1)
    t = np.ascontiguousarray(np.asarray(targets, dtype=np.float32)).reshape(N, 1)
    s = np.ascontiguousarray(np.asarray(spectators, dtype=np.float32)).reshape(N)
    in_maps = []
    for k in range(NCORES):
        idx = _row_index(k)
        in_maps.append({
            "x_full": x,
            "s_full": s,
            "x_strip": np.ascontiguousarray(x[idx]),
            "t_strip": np.ascontiguousarray(t[idx]),
            "s_strip": np.ascontiguousarray(s[idx]),
        })
    return in_maps


def _combine(results):
    """results: list of per-core dicts with partials/rowparts/colparts."""
    g = np.zeros(NOUT, np.float64)
    A = np.zeros(N, np.float64)
    B = np.zeros(N, np.float64)
    for k in range(NCORES):
        g += results[k]["partials"].astype(np.float64)
        rowpart = results[k]["rowparts"].astype(np.float64)  # [128, 16]
        idx = _row_index(k)
        A[idx] += rowpart[:, 0:8].T.reshape(-1)
        B[idx] += rowpart[:, 8:16].T.reshape(-1)
        colpart = results[k]["colparts"].astype(np.float64)  # [128, 7*16]
        cp = colpart.reshape(P, 7, 16)
        # col index (jt-1)*16 + q (A) / + 8 + q (B); j = jt*1024 + q*128 + p
        Ac = cp[:, :, 0:8].transpose(1, 2, 0).reshape(-1)   # [7*8*128] j-ordered
        Bc = cp[:, :, 8:16].transpose(1, 2, 0).reshape(-1)
        A[JT:] += Ac
        B[JT:] += Bc

    cnt, smp, smp2, sms, sms2, bce_sum = g[6], g[7], g[8], g[9], g[10], g[11]
    Sxy = g[5] + 2.0 * g[12]
    sAB = float(A @ B)
    sAA = float(A @ A)
    sBB = float(B @ B)
    Tx = float(A.sum())
    Ty = float(B.sum())

    bce = bce_sum / N
    c = max(cnt, 1.0)
    Sxx = 2.0 * c * smp2 - 2.0 * smp * smp
    Syy = 2.0 * c * sms2 - 2.0 * sms * sms
    Vxy = Sxy - (2.0 / c) * sAB + Tx * Ty / (c * c)
    Vxx = Sxx - (2.0 / c) * sAA + Tx * Tx / (c * c)
    Vyy = Syy - (2.0 / c) * sBB + Ty * Ty / (c * c)
    EPS = 1e-8
    dcov = np.sqrt(max(Vxy / (c * c), EPS))
    dvx = np.sqrt(max(Vxx / (c * c), EPS))
    dvy = np.sqrt(max(Vyy / (c * c), EPS))
    dcor = dcov / (dvx * dvy)
    loss = bce + (dcor if cnt > 0 else 0.0)
    return np.float32(loss)


def kernel(inputs, targets, spectators):
    nc = _get_nc()
    in_maps = _make_in_maps(inputs, targets, spectators)
    res = run_bass_kernel_spmd(nc, in_maps, list(range(NCORES)))
    return _combine(res.results)


if __name__ == "__main__":
    d = np.load("/root/problem/cached_io.npz")
    out = kernel(d["inputs"], d["targets"], d["spectators"])
    exp = float(d["expected"])
    rel = abs(float(out) - exp) / abs(exp)
    print(f"kernel: {float(out):.8f}  expected: {exp:.8f}  rel err: {rel:.3e}")



# revision 4
# speedup vs baseline: 6.0265x; 6.0265x over previous
"""Trainium2 Bass kernel for nn_ConditionalDisCoLoss.

loss = BCEWithLogits(inputs, targets)
     + dCor_masked(sigmoid(inputs), spectators, mask=spectators>=0.5)

Split of work:
  host (float64, exact): BCE, the O(n log n) sort-based closed forms for
    the masked row sums A_i = sum_j m_i m_j |p_i - p_j| (and B_i for s),
    Sxx/Syy closed forms, and the final dcov/dvar assembly.
  device (the O(c^2) core): Sxy = sum_ij m_i m_j |p_i-p_j||s_i-s_j|.

Device formulation: the product of differences is rank 4,
  (p_i-p_j)(s_i-s_j) = ps_i*1 + 1*ps_j - p_i*s_j - s_i*p_j,
so a single K=16 bf16 matmul (hi+lo split of each of the 4 row vectors
on both sides keeps ~2^-17 relative accuracy) produces
  D3[i,j] = m_i m_j (p_i-p_j)(s_i-s_j)
directly in PSUM, and the only elementwise work left is one
abs+row-sum-accumulate pass per tile, split between ACT (activation Abs
with accum_out) and DVE (tensor_reduce add with apply_absolute_value).

Compaction: only masked samples contribute (every term carries m_i m_j),
so the host compacts to the c masked entries (c ~ Binomial(8192, 1/2),
i.e. 4096 +- 45) padded with zeros to C = 5120 columns.  Rows are dealt
round-robin over 8 cores exactly like the full version: core k owns
i-tiles {8*it + k : it in 0..4}, each sitting in band it, and computes
j-bands jt >= it (15 [128 x 1024] tiles); same-band tile sums count
once, cross-band twice (both orientations of a same-band pair are
computed, cross-band pairs only once).
"""

import numpy as np
from contextlib import ExitStack

import ml_dtypes
import concourse.bass as bass
import concourse.bacc as bacc
import concourse.tile as tile
from concourse import mybir
from concourse.bass_utils import run_bass_kernel_spmd

N = 8192
NCORES = 8
C = 5120                 # padded capacity for masked entries (>= cnt w.h.p.)
P = 128
JT = 1024                # j-tile width = band width
NB = C // JT             # 5 bands
NIT = NB                 # i-tiles per core (core k owns global tile 8*it+k)
STRIP = NIT * P          # 640 rows per core
K16 = 16                 # matmul contraction: 4 terms x (hi,lo) x (hi,lo)
NCOL = 16                # rowparts columns (15 used)

BF16 = mybir.dt.bfloat16
F32 = mybir.dt.float32
ALU = mybir.AluOpType
ACTF = mybir.ActivationFunctionType
AX = mybir.AxisListType

# cost-model estimates (ns) used for the greedy ACT/DVE split
_ACT_TILE_NS = 1275.0
_DVE_TILE_NS = 1192.0


def _build():
    nc = bacc.Bacc("TRN2", target_bir_lowering=False, debug=False,
                   num_devices=NCORES, enable_asserts=False)

    Ld = nc.dram_tensor("L16", [K16, STRIP], BF16, kind="ExternalInput")
    Rd = nc.dram_tensor("R16", [K16, C], BF16, kind="ExternalInput")
    outd = nc.dram_tensor("rowparts", [P, NCOL], F32, kind="ExternalOutput")

    with tile.TileContext(nc) as tc, ExitStack() as ctx:
        pre = ctx.enter_context(tc.tile_pool(name="pre", bufs=1))
        junkp = ctx.enter_context(tc.tile_pool(name="junk", bufs=3))
        psp = ctx.enter_context(tc.tile_pool(name="ps", bufs=4, space="PSUM"))

        # stationary rows for this core's 640 gathered rows
        Ls = pre.tile([K16, STRIP], BF16)
        nc.sync.dma_start(out=Ls, in_=Ld.ap())

        # moving rows, chunked across engine DMA queues for parallel load
        Rs = pre.tile([K16, C], BF16)
        engs = [nc.scalar, nc.gpsimd, nc.sync]
        for b in range(NB):
            engs[b % len(engs)].dma_start(
                out=Rs[:, b * JT:(b + 1) * JT],
                in_=Rd.ap()[:, b * JT:(b + 1) * JT])

        Rp = pre.tile([P, NCOL], F32)
        nc.vector.memset(Rp, 0.0)

        act_load = 0.0
        dve_load = 0.0
        col = 0
        for it in range(NB):
            lT = Ls[:, it * P:(it + 1) * P]
            for jt in range(it, NB):
                ps = psp.tile([P, JT], F32, tag="ps")
                for h in range(2):
                    nc.tensor.matmul(ps[:, h * 512:(h + 1) * 512],
                                     lhsT=lT,
                                     rhs=Rs[:, jt * JT + h * 512:
                                            jt * JT + (h + 1) * 512],
                                     start=True, stop=True)
                if act_load + _ACT_TILE_NS <= dve_load + _DVE_TILE_NS:
                    junk = junkp.tile([P, JT], BF16, tag="junk")
                    nc.scalar.activation(junk, ps, ACTF.Abs,
                                         accum_out=Rp[:, col:col + 1])
                    act_load += _ACT_TILE_NS
                else:
                    nc.vector.tensor_reduce(Rp[:, col:col + 1], ps, AX.X,
                                            ALU.add,
                                            apply_absolute_value=True)
                    dve_load += _DVE_TILE_NS
                col += 1

        nc.sync.dma_start(out=outd.ap(), in_=Rp)

    nc.compile()
    return nc


_NC_CACHE = None


def _get_nc():
    global _NC_CACHE
    if _NC_CACHE is None:
        _NC_CACHE = _build()
    return _NC_CACHE


def _row_index(k):
    """Compacted row indices owned by core k (i-tiles 8*it + k)."""
    return np.concatenate([np.arange((8 * it + k) * P, (8 * it + k + 1) * P)
                           for it in range(NIT)])


def _hilo(v):
    hi = v.astype(ml_dtypes.bfloat16)
    lo = (v - hi.astype(np.float64)).astype(ml_dtypes.bfloat16)
    return hi, lo


def _masked_abs_sums(q_masked, cnt):
    """A_i = sum_j |q_i - q_j| over the masked set, exact via sorting."""
    order = np.argsort(q_masked, kind="stable")
    q = q_masked[order]
    pref = np.cumsum(q)
    tot = pref[-1]
    r = np.arange(1, cnt + 1, dtype=np.float64)
    aq = q * (2.0 * r - cnt) - (2.0 * pref - tot)
    a = np.empty(cnt, np.float64)
    a[order] = aq
    return a


def _prepare(inputs, targets, spectators):
    """Host preprocessing. Returns (in_maps, ctx) where ctx carries every
    host-side scalar needed by _finish."""
    x = np.asarray(inputs, np.float64).reshape(-1)
    t = np.asarray(targets, np.float64).reshape(-1)
    s = np.asarray(spectators, np.float64).reshape(-1)
    p = 1.0 / (1.0 + np.exp(-x))
    bce = float(np.mean(np.maximum(x, 0.0) - x * t
                        + np.log1p(np.exp(-np.abs(x)))))

    m = s >= 0.5
    cnt = int(m.sum())
    ctx = {"bce": bce, "cnt": cnt}
    if cnt == 0 or cnt > C:
        # cnt == 0: plain BCE. cnt > C (never for ~N(4096,45) but kept for
        # correctness): exact host fallback for Sxy.
        if cnt > C:
            pm, sm = p[m], s[m]
            sxy = 0.0
            for lo_ in range(0, cnt, 512):
                hi_ = min(lo_ + 512, cnt)
                dp = np.abs(pm[lo_:hi_, None] - pm[None, :])
                ds = np.abs(sm[lo_:hi_, None] - sm[None, :])
                sxy += float(np.sum(dp * ds))
            ctx["sxy_host"] = sxy
            ctx.update(_host_terms(p[m], s[m], cnt))
        return None, ctx

    pm, sm = p[m], s[m]
    ctx.update(_host_terms(pm, sm, cnt))

    pz = np.zeros(C, np.float64)
    sz = np.zeros(C, np.float64)
    mz = np.zeros(C, np.float64)
    pz[:cnt] = pm
    sz[:cnt] = sm
    mz[:cnt] = 1.0

    Lv = np.stack([mz * pz * sz, -mz * pz, -mz * sz, mz])   # [4, C]
    Rv = np.stack([mz, mz * sz, mz * pz, mz * pz * sz])     # [4, C]
    Lhi, Llo = _hilo(Lv)
    Rhi, Rlo = _hilo(Rv)
    L16 = np.concatenate([Lhi, Lhi, Llo, Llo], 0)           # [16, C]
    R16 = np.concatenate([Rhi, Rlo, Rhi, Rlo], 0)
    R16 = np.ascontiguousarray(R16)

    in_maps = []
    for k in range(NCORES):
        idx = _row_index(k)
        in_maps.append({"L16": np.ascontiguousarray(L16[:, idx]),
                        "R16": R16})
    return in_maps, ctx


def _host_terms(pm, sm, cnt):
    c = float(max(cnt, 1))
    A = _masked_abs_sums(pm, cnt)
    B = _masked_abs_sums(sm, cnt)
    smp = float(pm.sum())
    smp2 = float((pm * pm).sum())
    sms = float(sm.sum())
    sms2 = float((sm * sm).sum())
    return {
        "c": c,
        "Tx": float(A.sum()), "Ty": float(B.sum()),
        "sAB": float(A @ B), "sAA": float(A @ A), "sBB": float(B @ B),
        "Sxx": 2.0 * c * smp2 - 2.0 * smp * smp,
        "Syy": 2.0 * c * sms2 - 2.0 * sms * sms,
    }


def _finish(results, ctx):
    if ctx["cnt"] == 0:
        return np.float32(ctx["bce"])
    if "sxy_host" in ctx:
        sxy = ctx["sxy_host"]
    else:
        sxy = 0.0
        for k in range(NCORES):
            rp = results[k]["rowparts"].astype(np.float64)
            col = 0
            for it in range(NB):
                for jt in range(it, NB):
                    w = 1.0 if jt == it else 2.0
                    sxy += w * float(rp[:, col].sum())
                    col += 1
    c = ctx["c"]
    Vxy = sxy - (2.0 / c) * ctx["sAB"] + ctx["Tx"] * ctx["Ty"] / (c * c)
    Vxx = ctx["Sxx"] - (2.0 / c) * ctx["sAA"] + ctx["Tx"] ** 2 / (c * c)
    Vyy = ctx["Syy"] - (2.0 / c) * ctx["sBB"] + ctx["Ty"] ** 2 / (c * c)
    EPS = 1e-8
    dcov = np.sqrt(max(Vxy / (c * c), EPS))
    dvx = np.sqrt(max(Vxx / (c * c), EPS))
    dvy = np.sqrt(max(Vyy / (c * c), EPS))
    return np.float32(ctx["bce"] + dcov / (dvx * dvy))


def kernel(inputs, targets, spectators):
    in_maps, ctx = _prepare(inputs, targets, spectators)
    if in_maps is None:
        return _finish(None, ctx)
    nc = _get_nc()
    res = run_bass_kernel_spmd(nc, in_maps, list(range(NCORES)))
    return _finish(res.results, ctx)
